# revision 1
# baseline (speedup 1.0000x reference)
"""Trainium2 Bass kernel for nn_EnsembleModel2 (grouped tiny-GEMM + softmax-dot).

Math per (batch b, group g):
    y = x[b,g,:] @ W[g].T + bias[g]        # [64]
    resp = softmax(y)                      # over the 64 features
    out[b,g] = sum(resp * x[b,g,:])

Identity used on-device: softmax(y+bias).x summed ==
    (sum_m e^{y_m} * e^{bias_m} * x_m) / (sum_m e^{y_m} * e^{bias_m})
so the bias folds into the reduction weights (e^bias), letting the exp run
bias-free and batched.

Sharding: EXPERT-parallel — 46 groups per core (full 4096 batch). This keeps
the per-core x traffic identical to batch sharding (48 MB) but shrinks the
weight traffic 8x vs replication (1.5 MB/core block-diag stack).

Per-core pipeline, groups in pairs (2x64 features = 128 partitions), batch in
blocks of 512 columns; one "superblock" = one pair x 4 batch-blocks (1 MB x):
    matmul  Y.T[128,512] = Wblk[j].T @ X[:, blk]     (fp32r, full-rate)
    exp     E = exp(Y.T)                             (ScalarE, 2 blocks/op)
    mul     EX = E * X                               (VectorE/GpSimdE alternating)
    matmul  den[2,512] = S[j].T @ E                  (fp16, S = e^bias selector)
    matmul  num[2,512] = S[j].T @ EX
    4 batch-blocks pack into one PSUM bank (rows 32q..32q+1) ->
    recip+mul per superblock, 8-superblock staged output flush.
The reduce matmuls trail the mains by two half-blocks (software pipelining)
so the PE never idles waiting on exp/mul.
"""

import numpy as np

import concourse.bass as bass
import concourse.mybir as mybir
import concourse.tile as tile
from concourse import bacc
from concourse.bass_utils import run_bass_kernel_spmd

NCORES = 8
B = 4096
G = 368
NM = 64
GC = G // NCORES          # 46 groups per core
NPAIR = GC // 2           # 23 pairs per core
BBLK = 512                # batch columns per matmul
NBB = B // BBLK           # 8 batch blocks
SBP = 4                   # batch blocks per superblock
NSB = NPAIR * (NBB // SBP)  # 46 superblocks (pair, half-of-batch)
DEPTH = 3                 # software-pipeline depth (half-blocks)

F32 = mybir.dt.float32
F32R = mybir.dt.float32r
F16 = mybir.dt.float16


def build_nc(niter: int = 1):
    """Per-core program. niter>1 statically repeats the sweep (timing)."""
    nc = bacc.Bacc()

    # xd[sb=(pair,hb), p=(h,n), q, col] ; per-partition 8KB contiguous
    xd = nc.dram_tensor("xd", [NSB, 128, SBP, BBLK], F32R, kind="ExternalInput")
    wd = nc.dram_tensor("wd", [128, NPAIR, 128], F32R, kind="ExternalInput")
    sd = nc.dram_tensor("sd", [128, NPAIR, 2], F16, kind="ExternalInput")
    od = nc.dram_tensor("od", [NSB, 8, BBLK], F32, kind="ExternalOutput")

    with tile.TileContext(nc) as tc:
        with (
            tc.tile_pool(name="singles", bufs=1) as singles,
            tc.tile_pool(name="xpool", bufs=6) as xpool,
            tc.tile_pool(name="epool", bufs=4) as epool,
            tc.tile_pool(name="xxpool", bufs=4) as xxpool,
            tc.tile_pool(name="ypool", bufs=4, space="PSUM") as ypool,
            tc.tile_pool(name="dpool", bufs=2, space="PSUM") as dpool,
            tc.tile_pool(name="npool", bufs=2, space="PSUM") as npool,
            tc.tile_pool(name="fpool", bufs=2) as fpool,
        ):
            w_all = singles.tile([128, NPAIR, 128], F32R)
            s_all = singles.tile([128, NPAIR, 2], F16)
            # (first pair's W rides behind the first x slab, issued in sweep)

            def sweep(rep=0):
                stages = {}
                fifo = []

                def emit_reduce(sb, half):
                    st = stages[sb]
                    pair = sb // 2
                    dent, numt = st["den"], st["num"]
                    et, ext = st["et"][half], st["ext"][half]
                    for k in range(2):
                        s = 2 * half + k
                        nc.tensor.matmul(
                            dent[32 * s: 32 * s + 2, :], s_all[:, pair, :],
                            et[:, k, :], start=True, stop=True,
                            tile_position=(0, 32 * s),
                        )
                        nc.tensor.matmul(
                            numt[32 * s: 32 * s + 2, :], s_all[:, pair, :],
                            ext[:, k, :], start=True, stop=True,
                            tile_position=(0, 32 * s),
                        )
                    if half == 1:
                        out_stage = st["ostg"]
                        inv = fpool.tile([128, BBLK], F32, tag="inv")
                        nc.vector.reciprocal(inv, dent)
                        nc.vector.tensor_mul(
                            out_stage[:, sb % 8, :], numt, inv
                        )
                        # Flush 8 superblocks at a time. Useful rows are
                        # {32q, 32q+1 : q in 0..3}; one DMA per row-within-
                        # slot (two-level partition APs mis-read on DMA).
                        if sb % 8 == 7 or sb == NSB - 1:
                            nflush = sb % 8 + 1
                            sb0 = sb - nflush + 1
                            stg = out_stage.rearrange(
                                "(s r) k f -> s r k f", s=4
                            )
                            odr = od[sb0: sb + 1, :, :].rearrange(
                                "n (s r) f -> s r n f", r=2
                            )
                            for r01 in range(2):
                                nc.scalar.dma_start(
                                    out=odr[:, r01, :, :],
                                    in_=stg[:, r01, 0:nflush, :],
                                )
                        del stages[sb]

                out_stage = None
                for sb in range(NSB):
                    pair = sb // 2
                    if sb % 8 == 0:
                        out_stage = fpool.tile([128, 8, BBLK], F32, tag="ostg")
                    xs = xpool.tile([128, SBP, BBLK], F32R, tag="xs")
                    nc.sync.dma_start(out=xs, in_=xd[sb, :, :, :])
                    if rep == 0 and sb == 0:
                        # constants ride behind the first x slab
                        nc.sync.dma_start(out=w_all[:, 0:1, :], in_=wd[:, 0:1, :])
                        nc.sync.dma_start(out=s_all, in_=sd[:, :, :])
                        nc.sync.dma_start(
                            out=w_all[:, 1:NPAIR, :], in_=wd[:, 1:NPAIR, :]
                        )
                    dent = dpool.tile([128, BBLK], F32, tag="den")
                    numt = npool.tile([128, BBLK], F32, tag="num")
                    stages[sb] = {"den": dent, "num": numt, "et": {},
                                  "ext": {}, "ostg": out_stage}
                    for half in range(2):
                        et = epool.tile([128, 2, BBLK], F16, tag="et")
                        for k in range(2):
                            s = 2 * half + k
                            yt = ypool.tile([128, BBLK], F32, tag="yt")
                            nc.tensor.matmul(
                                yt, w_all[:, pair, :], xs[:, s, :],
                                start=True, stop=True,
                            )
                            nc.scalar.activation(
                                et[:, k, :], yt,
                                mybir.ActivationFunctionType.Exp,
                            )
                        ext = xxpool.tile([128, 2, BBLK], F16, tag="ext")
                        mul_eng = nc.vector if half == 0 else nc.gpsimd
                        mul_eng.tensor_mul(
                            ext[:, :, :], et[:, :, :],
                            xs[:, 2 * half: 2 * half + 2, :],
                        )
                        stages[sb]["et"][half] = et
                        stages[sb]["ext"][half] = ext
                        fifo.append((sb, half))
                        if len(fifo) > DEPTH:
                            emit_reduce(*fifo.pop(0))
                while fifo:
                    emit_reduce(*fifo.pop(0))

            for rep in range(niter):
                sweep(rep)

    nc.finalize()
    return nc


def prep_inputs(x, W, b):
    """Host-side repack into the device layouts (free for the HW metric)."""
    x = np.ascontiguousarray(x, dtype=np.float32)
    W = np.asarray(W, dtype=np.float32)
    b = np.asarray(b, dtype=np.float32)

    # xd[c][(j,hb), p=(h,n), q, col] = x[(4hb+q)*512+col, 46c+2j+h, n]
    xr = x.reshape(2, SBP, BBLK, NCORES, NPAIR, 2, NM)  # [hb,q,col,c,j,h,n]
    xd = np.ascontiguousarray(xr.transpose(3, 4, 0, 5, 6, 1, 2)).reshape(
        NCORES, NSB, 128, SBP, BBLK
    )

    # Block-diag weight stack, lhsT layout: Wblk[j][:64,:64] = W[2j].T etc.
    WT = W.transpose(0, 2, 1)  # [g, n, m]
    w_blk = np.zeros((G // 2, 128, 128), dtype=np.float32)
    w_blk[:, :NM, :NM] = WT[0::2]
    w_blk[:, NM:, NM:] = WT[1::2]
    # [c, 128, NPAIR, 128]
    wd = np.ascontiguousarray(
        w_blk.reshape(NCORES, NPAIR, 128, 128).transpose(0, 2, 1, 3)
    )

    # Reduction selector carrying e^bias
    eb = np.exp(b)  # [G, NM]
    s_red = np.zeros((G // 2, 128, 2), dtype=np.float32)
    s_red[:, :NM, 0] = eb[0::2]
    s_red[:, NM:, 1] = eb[1::2]
    sd = np.ascontiguousarray(
        s_red.reshape(NCORES, NPAIR, 128, 2).transpose(0, 2, 1, 3)
    ).astype(np.float16)

    return xd, wd, sd


def unpack_out(od_list):
    """od[c] is [NSB, 8, BBLK] = [(j,hb), (q,h), col];
    out[(4hb+q)*512+col, 46c+2j+h] = od[c][2j+hb, 2q+h, col]."""
    outs = []
    for od in od_list:
        o = od.reshape(NPAIR, 2, SBP, 2, BBLK)         # [j, hb, q, h, col]
        o = o.transpose(1, 2, 4, 0, 3).reshape(B, GC)  # [(hb,q,col), (j,h)]
        outs.append(o)
    return np.concatenate(outs, axis=1)  # concat along groups


_NC_CACHE = {}


def _get_nc(niter=1):
    if niter not in _NC_CACHE:
        _NC_CACHE[niter] = build_nc(niter)
    return _NC_CACHE[niter]


def kernel(x, W, b):
    import time as _time

    xd, wd, sd = prep_inputs(x, W, b)
    nc = _get_nc(1)
    in_maps = [
        {"xd": xd[c], "wd": wd[c], "sd": sd[c]} for c in range(NCORES)
    ]
    last_err = None
    for attempt in range(3):
        try:
            res = run_bass_kernel_spmd(nc, in_maps, core_ids=list(range(NCORES)))
            return unpack_out([res.results[c]["od"] for c in range(NCORES)])
        except Exception as e:  # transient NRT/tunnel failures; retry
            last_err = e
            _time.sleep(5.0 * (attempt + 1))
    raise last_err



# revision 29
# speedup vs baseline: 1.2440x; 1.2440x over previous
"""Trainium2 Bass kernel for nn_EnsembleModel2 (grouped tiny-GEMM + softmax-dot).

Math per (batch b, group g):
    y = x[b,g,:] @ W[g].T + bias[g]        # [64]
    resp = softmax(y)                      # over the 64 features
    out[b,g] = sum(resp * x[b,g,:])

Sharding: EXPERT-parallel - 46 groups per core (full 4096 batch), pairs of
groups packed into 128 partitions. x and W travel as fp16 (24 MB x per core).

Per-core pipeline, one superblock sb = (pair j, batch-half hb) = [128, 2048]:
    4x matmul   Y[128,2048] = Wblk[j].T @ X      (fp16, PSUM 4 banks)
    1x exp      E = exp(Y + bias[j])             (ScalarE, per-partition bias)
    1x mul      EX = E * X                       (VectorE, fp16 2x_1p mode)
    reduce      den/num via E/EX as the STATIONARY matmul operand against a
                constant [128,2] 0/1 group selector: out free size is 2, so
                the whole reduction costs ~100 ns of PE per superblock.
                den -> Y[:,0:32], num -> Y[:,32:64] (bank 0, reused post-exp)
    divide      out = num/den on GpSimd, staged 8 superblocks per output DMA.
The softmax denominator/numerator contract over partitions (features); batch
lands on the reduce-output partitions, so each 128-col chunk of a superblock
yields a [128, 2] slice of the result.
"""

import numpy as np

import concourse.bass as bass
import concourse.mybir as mybir
import concourse.tile as tile
from concourse import bacc
from concourse.bass_utils import run_bass_kernel_spmd

NCORES = 8
B = 4096
G = 368
NM = 64
GC = G // NCORES          # 46 groups per core
NPAIR = GC // 2           # 23 pairs per core
SBC = 2048                # batch columns per superblock (half the batch dim)
NSB = NPAIR * 2           # 46 superblocks = (pair, half)
NCH = SBC // 128          # 16 reduce chunks per superblock
FLUSH = 8                 # superblocks per output DMA
DN = SBC - 64             # den/num parking offset inside a y tile

F32 = mybir.dt.float32
F16 = mybir.dt.float16


def build_nc(niter: int = 1):
    """Per-core program. niter>1 statically repeats the sweep (timing)."""
    nc = bacc.Bacc()

    # xd[sb=(j,hb), p=(h,n), col] fp16; per-partition 4 KB contiguous
    xd = nc.dram_tensor("xd", [NSB, 128, SBC], F16, kind="ExternalInput")
    wd = nc.dram_tensor("wd", [128, NPAIR, 128], F16, kind="ExternalInput")
    bd = nc.dram_tensor("bd", [128, NPAIR], F32, kind="ExternalInput")
    sd = nc.dram_tensor("sd", [128, 2], F16, kind="ExternalInput")
    # od[p, 32*sb + 2*chunk + h] f32, partition-major for full-rate DMA
    od = nc.dram_tensor("od", [128, NSB * 32], F32, kind="ExternalOutput")

    with tile.TileContext(nc) as tc:
        with (
            tc.tile_pool(name="singles", bufs=1) as singles,
            tc.tile_pool(name="xpool", bufs=6) as xpool,
            tc.tile_pool(name="epool", bufs=3) as epool,
            tc.tile_pool(name="xxpool", bufs=3) as xxpool,
            tc.tile_pool(name="ypool", bufs=2, space="PSUM") as ypool,
            tc.tile_pool(name="spool", bufs=2) as spool,
            tc.tile_pool(name="ipool", bufs=2) as ipool,
        ):
            w_all = singles.tile([128, NPAIR, 128], F16)
            b_all = singles.tile([128, NPAIR], F32)
            s_t = singles.tile([128, 2], F16)

            # Dependency-free exp at t~0 absorbs the activation-table load
            # that would otherwise delay the first real exp.
            warm = singles.tile([128, 1], F32)
            warm_o = singles.tile([128, 1], F16)
            nc.vector.memset(warm, 0.0)
            nc.scalar.activation(warm_o, warm, mybir.ActivationFunctionType.Exp)

            def sweep(rep=0):
                stages = {}
                last_flush = [-1]

                def emit_reduce(sb, ytgt, dn=DN):
                    """Reduce + divide + flush for superblock sb.

                    den/num land in the tail columns of `ytgt` - the y tile of
                    superblock sb+1 (which the act of sb+1 has consumed by the
                    time these run). Routing them into the NEXT tile keeps the
                    release of sb's own y tile off the mul->reduce->divide
                    chain, so the mains two steps ahead never stall on it.
                    """
                    st = stages.pop(sb)
                    _, et, ext, stg = st
                    for c in range(NCH):
                        nc.tensor.matmul(
                            ytgt[:, dn + 2 * c: dn + 2 * c + 2],
                            et[:, 128 * c: 128 * (c + 1)], s_t,
                            start=True, stop=True,
                        )
                        nc.tensor.matmul(
                            ytgt[:, dn + 32 + 2 * c: dn + 32 + 2 * c + 2],
                            ext[:, 128 * c: 128 * (c + 1)], s_t,
                            start=True, stop=True,
                        )
                    # GPSIMD cannot touch PSUM, so num/den runs on DVE as
                    # reciprocal + multiply
                    inv = ipool.tile([128, 32], F32, tag="inv")
                    nc.vector.reciprocal(inv, ytgt[:, dn: dn + 32])
                    nc.vector.tensor_mul(
                        stg[:, sb % FLUSH, :], ytgt[:, dn + 32: dn + 64], inv
                    )
                    # flush early at NSB-2 so only one superblock's output
                    # DMA trails the last act
                    if sb % FLUSH == FLUSH - 1 or sb >= NSB - 2:
                        sb0 = last_flush[0] + 1
                        nflush = sb - sb0 + 1
                        last_flush[0] = sb
                        nc.sync.dma_start(
                            out=od[:, 32 * sb0: 32 * (sb + 1)],
                            in_=stg[:, sb0 % FLUSH: sb0 % FLUSH + nflush, :],
                        )

                stg = None
                for sb in range(NSB):
                    pair = sb // 2
                    if sb % FLUSH == 0:
                        stg = spool.tile([128, FLUSH, 32], F32, tag="stg")
                    if rep == 0 and sb == 0:
                        # w0 leads on the fast HWDGE path; b/s ride the Pool
                        # SWDGE path to keep HWDGE free for the first x slabs
                        nc.sync.dma_start(out=w_all[:, 0:1, :], in_=wd[:, 0:1, :])
                        nc.gpsimd.dma_start(out=b_all, in_=bd[:, :])
                        nc.gpsimd.dma_start(out=s_t, in_=sd[:, :])
                    xs = xpool.tile([128, SBC], F16, tag="xs")
                    yt = ypool.tile([128, SBC], F32, tag="yt")
                    et = epool.tile([128, SBC], F16, tag="et")
                    # split the first superblock so exp starts earlier in the
                    # pipeline fill
                    npiece = 2 if rep == 0 and sb == 0 else 1
                    for h in range(npiece):
                        lo, hi = h * SBC // npiece, (h + 1) * SBC // npiece
                        nc.sync.dma_start(out=xs[:, lo:hi], in_=xd[sb, :, lo:hi])
                        for k in range(lo // 512, hi // 512):
                            nc.tensor.matmul(
                                yt[:, 512 * k: 512 * (k + 1)], w_all[:, pair, :],
                                xs[:, 512 * k: 512 * (k + 1)],
                                start=True, stop=True,
                            )
                        nc.scalar.activation(
                            et[:, lo:hi], yt[:, lo:hi],
                            mybir.ActivationFunctionType.Exp,
                            bias=b_all[:, pair: pair + 1],
                        )
                    # staged weight prefetch: pairs 1-2 land before mains(2);
                    # the rest follows once the early x slabs are in flight
                    if rep == 0 and sb == 0:
                        nc.sync.dma_start(out=w_all[:, 1:3, :], in_=wd[:, 1:3, :])
                    if rep == 0 and sb == 3:
                        nc.sync.dma_start(out=w_all[:, 3:9, :], in_=wd[:, 3:9, :])
                    if rep == 0 and sb == 6:
                        nc.sync.dma_start(
                            out=w_all[:, 9:NPAIR, :], in_=wd[:, 9:NPAIR, :]
                        )
                    # reduce for sb-2 goes ahead of this superblock's mul so
                    # its recip/mul sit in front of the big mul in the DVE
                    # in-order queue (they gate the y-tile release)
                    if sb >= 2:
                        emit_reduce(sb - 2, stages[sb - 1][0])
                    # the mul always runs as two halves: the sb-2 recip/mul
                    # (which gate a y-tile release) can then slot in after
                    # half a mul instead of a full one on the in-order DVE
                    ext = xxpool.tile([128, SBC], F16, tag="ext")
                    nc.vector.tensor_mul(
                        ext[:, 0:SBC // 2], et[:, 0:SBC // 2], xs[:, 0:SBC // 2]
                    )
                    nc.vector.tensor_mul(
                        ext[:, SBC // 2:], et[:, SBC // 2:], xs[:, SBC // 2:]
                    )
                    stages[sb] = (yt, et, ext, stg)
                # drain: NSB-2's den/num go into yt(NSB-2) itself (that ypool
                # slot is never reused, and its act is already done), so the
                # 40-44 flush overlaps the last act; NSB-1 uses its own tile.
                yt_last = stages[NSB - 1][0]
                emit_reduce(NSB - 2, stages[NSB - 2][0])
                emit_reduce(NSB - 1, yt_last)

            for rep in range(niter):
                sweep(rep)

    nc.finalize()
    return nc


def prep_inputs(x, W, b):
    """Host-side repack into the device layouts (free for the HW metric)."""
    x = np.asarray(x, dtype=np.float32)
    W = np.asarray(W, dtype=np.float32)
    b = np.asarray(b, dtype=np.float32)

    # xd[c][sb=(j,hb), p=(h,n), col] = x[2048*hb+col, 46c+2j+h, n]
    xr = x.reshape(2, SBC, NCORES, NPAIR, 2, NM)  # [hb,col,c,j,h,n]
    xd = np.ascontiguousarray(
        xr.transpose(2, 3, 0, 4, 5, 1)            # [c,j,hb,h,n,col]
    ).reshape(NCORES, NSB, 128, SBC).astype(np.float16)

    # Block-diag weight stack, lhsT layout: Wblk[j][:64,:64] = W[2j].T etc.
    WT = W.transpose(0, 2, 1)  # [g, n, m]
    w_blk = np.zeros((G // 2, 128, 128), dtype=np.float32)
    w_blk[:, :NM, :NM] = WT[0::2]
    w_blk[:, NM:, NM:] = WT[1::2]
    wd = np.ascontiguousarray(
        w_blk.reshape(NCORES, NPAIR, 128, 128).transpose(0, 2, 1, 3)
    ).astype(np.float16)  # [c, 128, NPAIR, 128]

    # Per-partition bias for the exp: bd[c][64h+m, j] = b[46c+2j+h, m]
    br = b.reshape(NCORES, NPAIR, 2, NM)          # [c,j,h,m]
    bd = np.ascontiguousarray(br.transpose(0, 2, 3, 1)).reshape(
        NCORES, 128, NPAIR
    )

    # Constant 0/1 group-membership selector (reduce-matmul moving operand)
    sd = np.zeros((128, 2), dtype=np.float16)
    sd[:NM, 0] = 1.0
    sd[NM:, 1] = 1.0

    return xd, wd, bd, sd


def unpack_out(od_list):
    """od[c] is [128, NSB*32] f32 with col = 32*(2j+hb) + 2*chunk + h;
    out[2048*hb + 128*chunk + p, 46c+2j+h] = od[c][p, col]."""
    outs = []
    for o in od_list:
        o = o.reshape(128, NPAIR, 2, NCH, 2)       # [p, j, hb, ch, h]
        o = o.transpose(2, 3, 0, 1, 4).reshape(B, GC)  # [(hb,ch,p), (j,h)]
        outs.append(o)
    return np.concatenate(outs, axis=1)  # concat along groups


_NC_CACHE = {}


def _get_nc(niter=1):
    if niter not in _NC_CACHE:
        _NC_CACHE[niter] = build_nc(niter)
    return _NC_CACHE[niter]


def kernel(x, W, b):
    import time as _time

    xd, wd, bd, sd = prep_inputs(x, W, b)
    nc = _get_nc(1)
    in_maps = [
        {"xd": xd[c], "wd": wd[c], "bd": bd[c], "sd": sd}
        for c in range(NCORES)
    ]
    last_err = None
    for attempt in range(3):
        try:
            res = run_bass_kernel_spmd(nc, in_maps, core_ids=list(range(NCORES)))
            return unpack_out([res.results[c]["od"] for c in range(NCORES)])
        except Exception as e:  # transient NRT/tunnel failures; retry
            last_err = e
            _time.sleep(5.0 * (attempt + 1))
    raise last_err
